# revision 51
# baseline (speedup 1.0000x reference)
"""Trainium2 Bass kernel for nn_Attention_61168924229643.

4-head attention over 1024 tokens, dim_head=32, with the reference's quirks:
  - l2norm over the TOKEN axis (axis=1 of (B, HW, h, d)),
  - `attn - attn.argmax(-1)` before softmax: a per-row constant shift that
    cancels exactly inside jax.nn.softmax. Logits are bounded (|S| < 0.6),
    so a raw exp/sum softmax reproduces the reference to ~2e-5.

Sharding: B=8 batch elements -> one NeuronCore each, no collectives.

Layout: tokens on the SBUF free axis, channels on partitions ("transposed").
x arrives host-transposed/bf16-cast; attention is permutation-equivariant
over tokens and a permuted token order (token 8p+t <-> column t*128+p) makes
both the input and output DMAs contiguous per partition.

Performance structure (v4):
  - DMA cost here is ~42ns per partition-row packet, so [128, n] transfers
    are ~5.4us regardless of width. All critical inputs (xt, w_qkv, w_out
    tiles) are packed into ONE bf16 DRAM tensor and loaded as 3
    partition-range slices on the 3 DMA-capable queues (~2.5us landing).
    The ktbd zero background is DVE/GPSIMD memsets instead of DMA; the
    (zero) bias load is deferred into phase B; output DMAs are split by
    partition range across queues.
  - No blind PE warm-up block: real QKV matmuls + a few dummy matmuls warm
    the HAM clock-gate, and the phases keep the PE nearly saturated so it
    never re-throttles to half clock.
  - Head-major two-phase loop: heads {0,1} accumulate into o_a and finish
    before heads {2,3} start into o_b; o_a's normalization/projection
    overlap phase B, so only o_b's epilogue is a serial tail.
  - PV pairing: the two heads of a phase write disjoint 64-column groups
    of the PE array (tile_position (0,0)/(0,64)); adjacent emission makes
    them run concurrently => PV is ~1024 cycles per phase-jt.
  - S-tile triple buffering: the S/exp pipeline borrows the idle
    accumulator banks of the other phase (v_ps banks during phase A,
    o_a/y banks mid-phase-B) so S(jt+1) hides under exp(jt).
  - exp split across engines: ACT computes true exp for most tiles; the
    odd head of each phase on QUAD_A/QUAD_B jts is computed on the Vector
    engine as a fitted quadratic E' = c*(x+a)^2 (t = sqrt(c)*x+sqrt(c)*a
    from PSUM, then E' = t*t in bf16 at 2x mode). The constant term c*b of
    the fit c*((x+a)^2+b) ~ exp(x) is folded into the PV accumulation as a
    rank-1 correction matmul computed on-device from V, and softmax is
    invariant to the per-head scale c. End-to-end adds ~2e-3.
  - Token-axis l2norm scales fold into one per-(h,d) factor
    s = 10/(||q|| ||k||) applied to Q by an ACT Copy with per-partition
    scale; norms via ACT Square accumulation + a compressed DVE bit-hack
    rsqrt on the combined product.
  - S matmuls use block-diagonal K stationaries (one head's 32 rows live,
    rest zero); softmax denominators ride as `ones` rows of the [V|1] PV
    stationaries; normalization uses reciprocal_approx_fast + partition
    remap DMAs into a background-1.0 tile, junk rows killed by zero rows
    of the zero-padded per-head-pair w_out inputs.
"""

import os
import numpy as np
import ml_dtypes
from contextlib import ExitStack

import concourse.tile as tile
from concourse import bacc, mybir
from concourse.bass_utils import run_bass_kernel_spmd

FP32 = mybir.dt.float32
BF16 = mybir.dt.bfloat16

HW = 1024          # tokens per batch element (32*32)
C = 128            # channels
HEADS = 4
DH = 32            # dim per head
N_CORES = 8
NT = HW // 128     # 8 token tiles

# packed input layout (bf16 columns): [xt 1024 | wqkv 384 | woa 128 | wob 128]
INP_W = 1024 + 3 * C + C + C

# Quadratic exp fit: c*((x+a)^2 + b) ~ exp(x) on the logit range [-0.65, 0.55]
QA = 1.106669
QB = 0.949980
QC = 0.461088
SQC = QC ** 0.5          # folded into the DVE pass so E' = c*(x+a)^2
CB = QC * QB             # constant term, folded into PV via corr matmul

# jt values (per phase) whose ODD head S-tile is computed on the DVE
# quadratic instead of ACT exp.
QUAD_A = tuple(
    int(t) for t in os.environ.get("QUADA", "1,2,4,6,7").split(",") if t != ""
)
QUAD_B = tuple(
    int(t) for t in os.environ.get("QUADB", "2,4,6").split(",") if t != ""
)
N_DUM_PRE = int(os.environ.get("DUMPRE", "6"))
N_DUM_MID = int(os.environ.get("DUMMID", "7"))
DUM_ANCHOR = os.environ.get("DUMANCHOR", "1") == "1"
N_DUM_POST = int(os.environ.get("DUMPOST", "0"))
N_DUM_SEAM = int(os.environ.get("DUMSEAM", "0"))
# phase-B st-tile borrow positions (indices into the 16-tile stream) taken
# from the o_a/y banks once o_a's epilogue has read them; phase A borrows
# every 3rd tile from the (idle) v banks.
BORROW_B = tuple(
    int(t) for t in os.environ.get("BORROWB", "6,9,12").split(",") if t != ""
)


def build_kernel_body(ctx, tc, out_d, inp_d, bias_d):
    nc = tc.nc
    Exp = mybir.ActivationFunctionType.Exp
    Square = mybir.ActivationFunctionType.Square
    Copy = mybir.ActivationFunctionType.Copy
    mult = mybir.AluOpType.mult
    add = mybir.AluOpType.add
    shr = mybir.AluOpType.logical_shift_right

    const = ctx.enter_context(tc.tile_pool(name="const", bufs=1))
    sb = ctx.enter_context(tc.tile_pool(name="sb", bufs=1))
    tqp = ctx.enter_context(tc.tile_pool(name="tqp", bufs=2))
    # PSUM: stp rotates 2x 4KB/partition tiles (2 banks each);
    # ops (o_a -> phase-B st borrow -> y) and rps (v -> phase-A st borrow
    # -> o_b) 2 banks each. 4 + 2 + 2 = 8 banks.
    stp = ctx.enter_context(tc.tile_pool(name="stp", bufs=2, space="PSUM"))
    ops = ctx.enter_context(tc.tile_pool(name="ops", bufs=1, space="PSUM"))
    rps = ctx.enter_context(tc.tile_pool(name="rps", bufs=1, space="PSUM"))

    # ---- ACT table warm-up: touch Exp and Square immediately so the table
    # load overlaps the input DMAs instead of stalling the prologue.
    warm = const.tile([128, 1], FP32, tag="warm")
    nc.vector.memset(warm[:], 1.0)
    warm2 = const.tile([128, 1], FP32, tag="warm2")
    nc.scalar.activation(warm2[:], warm[:], Exp)
    nc.scalar.activation(warm2[:], warm[:], Square)

    # ---- packed input: ONE full-width DMA on sync. Measured: a single
    # [128, w] HWDGE DMA completes in ~3.6us end-to-end, while splitting it
    # across queues or partition ranges makes every in-flight DMA pile up
    # to 10-15us (packet-level round-robin + per-DMA completion receipt).
    inp = sb.tile([128, INP_W], BF16, tag="inp")
    nc.sync.dma_start(inp[:, :], inp_d[:, :])
    xtb = inp[:, 0:1024]
    wqb = inp[:, 1024:1024 + 3 * C]
    woa = inp[:, 1024 + 3 * C:1024 + 4 * C]
    wob = inp[:, 1024 + 4 * C:1024 + 5 * C]

    # ---- DVE/GPSIMD-built constants & backgrounds
    # wmm: 1.0s; dummy-matmul operand, ones-column for sum-of-V matmuls,
    # ones-row moving operand of corr matmuls. First so dummies start early.
    wmm = const.tile([128, 512], BF16, tag="wmm")
    nc.vector.memset(wmm[:], 1.0)
    # ktbd zero background: two memsets on DVE, two on GPSIMD
    ktbd = sb.tile([128, HEADS, 1024], BF16, tag="ktbd")
    nc.vector.memset(ktbd[:, 0, :], 0.0)
    nc.vector.memset(ktbd[:, 1, :], 0.0)
    nc.gpsimd.memset(ktbd[:, 2, :], 0.0)
    nc.gpsimd.memset(ktbd[:, 3, :], 0.0)
    # vb2[(j%128), t, h, 0:32] = V rows, [..., 32:64] = 1.0 (denominator)
    vb2 = sb.tile([128, NT, HEADS, 2 * DH], BF16, tag="vb2")
    nc.vector.memset(vb2[:, :, :, DH:2 * DH], 1.0)
    corr_sb = const.tile([1, 256], BF16, tag="corr_sb")
    nc.vector.memset(corr_sb[:], 0.0)

    bias = const.tile([128, NT, C], FP32, tag="bias")

    # PSUM persistent tiles. kt before qt so the sv tile (3rd stp
    # allocation) lands on kt's buffer (free early), not qt's (read late).
    kt_ps = stp.tile([128, 1024], FP32, tag="st", name="kt")
    qt_ps = stp.tile([128, 1024], FP32, tag="st", name="qt")
    o_a = ops.tile([128, 1024], FP32, tag="oacc", name="o_a")
    v_ps = rps.tile([128, 1024], FP32, tag="vacc", name="v_ps")

    # Prologue PE stream pinned into one dependency chain so the
    # scheduler's DMA-timing model cannot reorder it.
    pe_prev = [None]

    def pe_pin(bi):
        if pe_prev[0] is not None:
            tile.add_dep_helper(bi.ins, pe_prev[0].ins,
                                reason="pin prologue PE order")
        pe_prev[0] = bi

    def dummy_mm(n, target):
        for _ in range(n):
            pe_pin(nc.tensor.matmul(
                target[:, 0:512], lhsT=wmm[:, 0:128], rhs=wmm[:],
                start=True, stop=True, skip_group_check=True,
            ))

    # ---- pre-data dummies: PE activity while the input DMAs land, so the
    # HAM clock-gate ramp starts as early as possible.
    dummy_mm(N_DUM_PRE, o_a)

    # ---- Q^T then K^T then V
    for ih in range(2):
        pe_pin(nc.tensor.matmul(
            qt_ps[:, ih * 512:(ih + 1) * 512],
            lhsT=wqb[:, 0:C],
            rhs=xtb[:, ih * 512:(ih + 1) * 512],
            start=True, stop=True,
        ))
    for ih in range(2):
        pe_pin(nc.tensor.matmul(
            kt_ps[:, ih * 512:(ih + 1) * 512],
            lhsT=wqb[:, C:2 * C],
            rhs=xtb[:, ih * 512:(ih + 1) * 512],
            start=True, stop=True,
        ))
    for t in range(NT):
        pe_pin(nc.tensor.matmul(
            v_ps[:, t * 128:(t + 1) * 128],
            lhsT=xtb[:, t * 128:(t + 1) * 128],
            rhs=wqb[:, 2 * C:3 * C],
            start=True, stop=True,
        ))

    # ---- norm chain, spread across ACT and DVE ----
    # DVE: K^T to bf16 (source for the stripe DMAs) — emitted FIRST so the
    # scheduler runs it as soon as the K matmuls land, in parallel with the
    # ACT Squares, and it does not sit between ksq and the rsq chain.
    ktb = sb.tile([128, 1024], BF16, tag="ktb")
    nc.vector.tensor_copy(ktb[:], kt_ps[:])
    # ACT: Squares with free-axis accumulation -> nsq = [sum q^2, sum(.1k)^2]
    nsq = sb.tile([128, 2], FP32, tag="nsq")
    sq_scr = sb.tile([128, 1024], FP32, tag="sq_scr")
    sq1_i = nc.scalar.activation(sq_scr[:], qt_ps[:], Square,
                                 accum_out=nsq[:, 0:1])
    sq2_i = nc.scalar.activation(sq_scr[:], kt_ps[:], Square, scale=0.1,
                                 accum_out=nsq[:, 1:2])

    # combined scale = rsqrt(nq2 * 0.01*nk2) = 10/(||q|| ||k||) via the
    # fp32 bit-hack + 1 Newton step (compressed: one chain on the product).
    chain_is = []
    m0 = sb.tile([128, 1], FP32, tag="m0")
    chain_is.append(nc.vector.tensor_mul(m0[:], nsq[:, 0:1], nsq[:, 1:2]))
    mi = m0[:].bitcast(mybir.dt.int32)
    yi = sb.tile([128, 1], mybir.dt.int32, tag="yi")
    chain_is.append(nc.vector.tensor_scalar(yi[:], mi, 1, None, op0=shr))
    chain_is.append(nc.vector.tensor_scalar(yi[:], yi[:], -1, 0x5F3759DF,
                                            op0=mult, op1=add))
    y = yi[:].bitcast(FP32)
    t1 = sb.tile([128, 1], FP32, tag="t1")
    chain_is.append(nc.vector.tensor_mul(t1[:], y, y))
    chain_is.append(nc.vector.tensor_mul(t1[:], t1[:], m0[:]))
    chain_is.append(nc.vector.tensor_scalar(t1[:], t1[:], -0.5, 1.5,
                                            op0=mult, op1=add))
    rcomb = sb.tile([128, 1], FP32, tag="rcomb")
    rcomb_i = nc.vector.tensor_mul(rcomb[:], y, t1[:])
    chain_is.append(rcomb_i)

    # DVE: V scatter for the odd heads (the sum-of-V matmuls need them);
    # pinned behind the chain so it can't interleave into its sem gaps.
    cast_h13 = nc.vector.tensor_copy(
        vb2[:, :, 1::2, 0:DH],
        v_ps[:].rearrange("p (t h d) -> p t h d", t=NT, h=HEADS)[:, :, 1::2, :],
    )
    tile.add_dep_helper(cast_h13.ins, rcomb_i.ins, reason="after rsq chain")

    # block-diagonal K stripes via SBUF->SBUF DMA (h0 first: gates S(0,0))
    nc.sync.dma_start(ktbd[0:32, 0, :], ktb[0:32, :])
    nc.gpsimd.dma_start(ktbd[32:64, 1, :], ktb[32:64, :])
    nc.sync.dma_start(ktbd[64:96, 2, :], ktb[64:96, :])
    nc.gpsimd.dma_start(ktbd[96:128, 3, :], ktb[96:128, :])

    # mid-prologue dummies keep the PE busy through the norm chain; each is
    # tied to a norm-chain event so leftovers can never queue up in front
    # of the first real S matmuls (the PE pops ready work by priority).
    anchors = [sq1_i, sq1_i, sq1_i, sq2_i,
               chain_is[2], chain_is[4], chain_is[6]]
    for di in range(N_DUM_MID):
        bi = nc.tensor.matmul(
            o_a[:, 0:512], lhsT=wmm[:, 0:128], rhs=wmm[:],
            start=True, stop=True, skip_group_check=True,
        )
        pe_pin(bi)
        if DUM_ANCHOR:
            anchor = anchors[min(di, len(anchors) - 1)]
            tile.add_dep_helper(bi.ins, anchor.ins,
                                reason="trickle with chain")

    # ---- sum-of-V for the quadratic correction (odd head of each phase,
    # over that phase's QUAD tiles): sv[0, h*64+m] = sum_j vb2[j, jt, h, m]
    sv_ps = stp.tile([128, 1024], FP32, tag="st", name="sv")
    first_sv = True
    for h, quad in ((1, QUAD_A), (3, QUAD_B)):
        for jt in quad:
            pe_pin(nc.tensor.matmul(
                sv_ps[0:1, h * 64:(h + 1) * 64],
                lhsT=wmm[:, 0:1],
                rhs=vb2[:, jt, h, :],
                start=first_sv, stop=False,
                skip_group_check=True,
            ))
            first_sv = False
    if QUAD_A:
        nc.vector.tensor_scalar_mul(corr_sb[0:1, 64:128],
                                    sv_ps[0:1, 64:128], CB)
    if QUAD_B:
        nc.vector.tensor_scalar_mul(corr_sb[0:1, 192:256],
                                    sv_ps[0:1, 192:256], CB)

    # post-norm-chain dummies: the PE would otherwise idle for the rest of
    # the norm chain (rsq + qtb) and the HAM clock-gate would re-throttle
    # right as the S-pass starts.
    dummy_mm(N_DUM_POST, o_a)

    # Q scaled by the combined factor, on ACT, split in column halves so
    # S(0,0) ih0 can start after the first half.
    qtb = sb.tile([128, 1024], BF16, tag="qtb")
    nc.scalar.activation(qtb[:, 0:512], qt_ps[:, 0:512], Copy,
                         scale=rcomb[:, 0:1])
    nc.scalar.activation(qtb[:, 512:1024], qt_ps[:, 512:1024], Copy,
                         scale=rcomb[:, 0:1])

    # DVE: remaining V scatter (even heads) and the 1.0 backgrounds for
    # the normalization remaps — pinned behind the rsq chain so the
    # scheduler cannot interleave them into its semaphore gaps.
    cast_h02 = nc.vector.tensor_copy(
        vb2[:, :, 0::2, 0:DH],
        v_ps[:].rearrange("p (t h d) -> p t h d", t=NT, h=HEADS)[:, :, 0::2, :],
    )
    tile.add_dep_helper(cast_h02.ins, rcomb_i.ins, reason="after rsq chain")
    rash = sb.tile([128, 1024], FP32, tag="rash")
    ms1 = nc.vector.memset(rash[:], 1.0)
    tile.add_dep_helper(ms1.ins, rcomb_i.ins, reason="after rsq chain")
    rbsh = sb.tile([128, 1024], FP32, tag="rbsh")
    ms2 = nc.vector.memset(rbsh[:], 1.0)
    tile.add_dep_helper(ms2.ins, rcomb_i.ins, reason="after rsq chain")

    eb_a = sb.tile([128, NT, 2, 1024], BF16, tag="eb_a")
    eb_b = sb.tile([128, NT, 2, 1024], BF16, tag="eb_b")
    ra = sb.tile([128, 1024], FP32, tag="ra")
    rb = sb.tile([128, 1024], FP32, tag="rb")
    stack_a = sb.tile([128, 1024], BF16, tag="stack_a")
    stack_b = sb.tile([128, 1024], BF16, tag="stack_b")
    yout = sb.tile([128, NT, C], FP32, tag="yout")
    out_v = out_d.rearrange("(p t) c -> p t c", p=128)
    y_holder = [None]

    def emit_s(st, jt, h):
        for ih in range(2):
            nc.tensor.matmul(
                st[:, ih * 512:(ih + 1) * 512],
                lhsT=ktbd[:, h, jt * 128:(jt + 1) * 128],
                rhs=qtb[:, ih * 512:(ih + 1) * 512],
                start=True, stop=True,
            )

    def emit_quad(st, eb, jt):
        tq = tqp.tile([128, 1024], BF16, tag="tq")
        nc.vector.tensor_scalar(tq[:], st[:], SQC, SQC * QA,
                                op0=mult, op1=add)
        nc.vector.tensor_mul(eb[:, jt, 1, :], tq[:], tq[:])

    def emit_corr(o, hp):
        # opens the accumulation group: start=True clears each bank's
        # has_written, then writes the rank-1 quadratic correction.
        for ih in range(2):
            pe_pin(nc.tensor.matmul(
                o[:, ih * 512:(ih + 1) * 512],
                lhsT=corr_sb[0:1, hp * 128:(hp + 1) * 128],
                rhs=wmm[0:1, :],
                start=True, stop=False,
                skip_group_check=True,
            ))

    def emit_pv(o, eb, hp, jt, ih_order=(0, 1)):
        # two heads in disjoint PE column groups, emitted adjacently so
        # the hardware runs them concurrently.
        for ih in ih_order:
            for hh in range(2):
                nc.tensor.matmul(
                    o[64 * hh:64 * hh + 64, ih * 512:(ih + 1) * 512],
                    lhsT=vb2[:, jt, 2 * hp + hh, :],
                    rhs=eb[:, jt, hh, ih * 512:(ih + 1) * 512],
                    start=False, stop=(jt == NT - 1),
                    tile_position=(0, 64 * hh),
                    skip_group_check=True,
                )

    def emit_phase(hp, o, eb, quad, st_alloc):
        for jt in range(NT):
            st0 = st_alloc(2 * jt)
            emit_s(st0, jt, 2 * hp)
            st1 = st_alloc(2 * jt + 1)
            emit_s(st1, jt, 2 * hp + 1)
            if jt == 0:
                # first tile of the phase: exp in column halves so ACT can
                # start as soon as the first S matmul lands (ACT paces the
                # whole pipeline; this pulls the phase start ~1us earlier).
                nc.scalar.activation(eb[:, 0, 0, 0:512], st0[:, 0:512], Exp)
                nc.scalar.activation(eb[:, 0, 0, 512:1024],
                                     st0[:, 512:1024], Exp)
            else:
                nc.scalar.activation(eb[:, jt, 0, :], st0[:], Exp)
            if jt in quad:
                emit_quad(st1, eb, jt)
            elif jt == 0:
                nc.scalar.activation(eb[:, 0, 1, 0:512], st1[:, 0:512], Exp)
                nc.scalar.activation(eb[:, 0, 1, 512:1024],
                                     st1[:, 512:1024], Exp)
            else:
                nc.scalar.activation(eb[:, jt, 1, :], st1[:], Exp)
            if jt == 0:
                # corr opens the accumulation group; emitted after jt0's S
                # tiles so it cannot delay the phase's pipeline restart
                # (it is only needed before the first PV, at jt1).
                emit_corr(o, hp)
            if jt > 0:
                emit_pv(o, eb, hp, jt - 1)
        # the final PV emits column-half 1 first so the epilogue's
        # second-half recip/remap chain starts as early as the first's.
        emit_pv(o, eb, hp, NT - 1, ih_order=(1, 0) if hp == 1 else (0, 1))

    def st_alloc_a(i):
        # every 3rd S-tile borrows the v_ps banks (idle during phase A)
        if i % 3 == 2:
            return rps.tile([128, 1024], FP32, tag="vacc", name=f"stA_{i}")
        return stp.tile([128, 1024], FP32, tag="st", name=f"stA_{i}")

    def st_alloc_b(i):
        if i in BORROW_B:
            return ops.tile([128, 1024], FP32, tag="oacc", name=f"stB_{i}")
        return stp.tile([128, 1024], FP32, tag="st", name=f"stB_{i}")

    def emit_remaps(rr, rsh, cs, flip):
        # partition remap of the denominator reciprocals on the two free
        # DMA queues (SBUF->SBUF, cheap).
        q0 = nc.sync if not flip else nc.gpsimd
        q1 = nc.gpsimd if not flip else nc.sync
        q0.dma_start(rsh[0:32, cs], rr[32:64, cs])
        q1.dma_start(rsh[64:96, cs], rr[96:128, cs])

    def emit_epilogue(hp, o, rr, rsh, stack, w_t, proj=True):
        # column halves; all recips + remap DMAs are issued up-front so the
        # two halves' remap completion receipts (~2.4us each) overlap
        # instead of chaining serially into the tail.
        w = 512
        outq = (nc.gpsimd, nc.sync)
        for ch in range(2):
            cs = slice(ch * w, (ch + 1) * w)
            nc.vector.reciprocal_approx_fast(rr[:, cs], o[:, cs])
            emit_remaps(rr, rsh, cs, flip=(ch % 2 == 1))
        for ch in range(2):
            cs = slice(ch * w, (ch + 1) * w)
            nc.vector.tensor_mul(stack[:, cs], o[:, cs], rsh[:, cs])
            if proj:
                emit_proj(hp, stack, w_t, ch, w // 128, outq[ch])

    def emit_proj(hp, stack, w_t, ch, nit, outq=None):
        if y_holder[0] is None:
            y_holder[0] = ops.tile([128, 1024], FP32, tag="oacc",
                                   name="y_ps")
        y_ps = y_holder[0]
        for it in range(ch * nit, ch * nit + nit):
            nc.tensor.matmul(
                y_ps[:, it * 128:(it + 1) * 128],
                lhsT=stack[:, it * 128:(it + 1) * 128],
                rhs=w_t[:],
                start=(hp == 0 and it % 4 == 0),
                stop=(hp == 1 and it % 4 == 3),
                skip_group_check=True,
            )
        if hp == 1:
            y_v = y_ps[:].rearrange("p (t c) -> p t c", t=NT)
            t0, t1 = ch * nit, ch * nit + nit
            nc.vector.tensor_add(yout[:, t0:t1, :], y_v[:, t0:t1, :],
                                 bias[:, t0:t1, :])
            outq.dma_start(out_v[:, t0:t1, :], yout[:, t0:t1, :])

    emit_phase(0, o_a, eb_a, QUAD_A, st_alloc_a)
    # phase A epilogue: recip/remap/mul overlap phase B; the projection is
    # emitted after phase B's borrowed tiles so the y banks are free.
    emit_epilogue(0, o_a, ra, rash, stack_a, woa, proj=False)
    o_b = rps.tile([128, 1024], FP32, tag="vacc", name="o_b")
    # seam dummies: PE filler while phase B's pipeline refills
    dummy_mm(N_DUM_SEAM, o_b)
    # deferred (zero) bias load: single sync DMA during phase B, pinned
    # behind the seam so the scheduler cannot float it into the prologue
    # where it would interfere with the critical input DMA.
    bias_i = nc.sync.dma_start(bias[:], bias_d[:])
    tile.add_dep_helper(bias_i.ins, pe_prev[0].ins,
                        reason="defer bias load past the seam")
    emit_phase(1, o_b, eb_b, QUAD_B, st_alloc_b)
    emit_proj(0, stack_a, woa, 0, 4)
    emit_proj(0, stack_a, woa, 1, 4)
    emit_epilogue(1, o_b, rb, rbsh, stack_b, wob, proj=True)


def build_nc():
    nc = bacc.Bacc("TRN2", target_bir_lowering=False, debug=False,
                   num_devices=N_CORES)
    inp_d = nc.dram_tensor("inp", [128, INP_W], BF16,
                           kind="ExternalInput").ap()
    bias_d = nc.dram_tensor("bias", [128, NT, C], FP32,
                            kind="ExternalInput").ap()
    out_d = nc.dram_tensor("out", [HW, C], FP32, kind="ExternalOutput").ap()
    with tile.TileContext(nc) as tc:
        with ExitStack() as ctx:
            build_kernel_body(ctx, tc, out_d, inp_d, bias_d)
    nc.compile()
    return nc


_CACHED_NC = None


def get_nc():
    global _CACHED_NC
    if _CACHED_NC is None:
        _CACHED_NC = build_nc()
    return _CACHED_NC


def make_in_maps(x, w_qkv, w_out, b_out):
    x = np.ascontiguousarray(np.asarray(x, dtype=np.float32)).reshape(N_CORES, HW, C)
    # [c, (t, p)] with column t*128+p = token 8p+t, bf16
    xt = np.ascontiguousarray(
        x.reshape(N_CORES, 128, NT, C).transpose(0, 3, 2, 1).reshape(N_CORES, C, HW)
    ).astype(ml_dtypes.bfloat16)
    w_qkv_bf = np.asarray(w_qkv, dtype=np.float32).astype(ml_dtypes.bfloat16)
    w_out = np.asarray(w_out, dtype=np.float32)
    b_out = np.asarray(b_out, dtype=np.float32).reshape(C)

    # woa: rows [w_out[0:32]; 0; w_out[32:64]; 0]  (heads 0, 1)
    # wob: rows [w_out[64:96]; 0; w_out[96:128]; 0]  (heads 2, 3)
    woa = np.zeros((128, C), dtype=np.float32)
    wob = np.zeros((128, C), dtype=np.float32)
    woa[0:32] = w_out[0:32]
    woa[64:96] = w_out[32:64]
    wob[0:32] = w_out[64:96]
    wob[64:96] = w_out[96:128]
    woa = woa.astype(ml_dtypes.bfloat16)
    wob = wob.astype(ml_dtypes.bfloat16)
    bias = np.ascontiguousarray(
        np.broadcast_to(b_out[None, None, :], (128, NT, C)).astype(np.float32))
    inp = np.concatenate(
        [xt, np.broadcast_to(w_qkv_bf[None], (N_CORES, C, 3 * C)),
         np.broadcast_to(woa[None], (N_CORES, 128, C)),
         np.broadcast_to(wob[None], (N_CORES, 128, C))], axis=2)
    inp = np.ascontiguousarray(inp).astype(ml_dtypes.bfloat16)
    return [
        {"inp": inp[i], "bias": bias}
        for i in range(N_CORES)
    ]


def kernel(x, w_qkv, w_out, b_out, _trace=False, _trace_kwargs=None):
    nc = get_nc()
    in_maps = make_in_maps(x, w_qkv, w_out, b_out)
    res = run_bass_kernel_spmd(
        nc, in_maps, core_ids=list(range(N_CORES)),
        trace=_trace, **(_trace_kwargs or {}),
    )
    out = np.stack([np.asarray(res.results[i]["out"]) for i in range(N_CORES)])
    out = out.reshape(8, 32, 32, 128).astype(np.float32)
    if _trace:
        kernel.last_result = res
    return out


# revision 52
# speedup vs baseline: 1.0349x; 1.0349x over previous
"""Trainium2 Bass kernel for nn_Attention_61168924229643.

4-head attention over 1024 tokens, dim_head=32, with the reference's quirks:
  - l2norm over the TOKEN axis (axis=1 of (B, HW, h, d)),
  - `attn - attn.argmax(-1)` before softmax: a per-row constant shift that
    cancels exactly inside jax.nn.softmax. Logits are bounded (|S| < 0.6),
    so a raw exp/sum softmax reproduces the reference to ~2e-5.

Sharding: B=8 batch elements -> one NeuronCore each, no collectives.

Layout: tokens on the SBUF free axis, channels on partitions ("transposed").
x arrives host-transposed/bf16-cast; attention is permutation-equivariant
over tokens and a permuted token order (token 8p+t <-> column t*128+p) makes
both the input and output DMAs contiguous per partition.

Performance structure (v4):
  - DMA cost here is ~42ns per partition-row packet, so [128, n] transfers
    are ~5.4us regardless of width. All critical inputs (xt, w_qkv, w_out
    tiles) are packed into ONE bf16 DRAM tensor and loaded as 3
    partition-range slices on the 3 DMA-capable queues (~2.5us landing).
    The ktbd zero background is DVE/GPSIMD memsets instead of DMA; the
    (zero) bias load is deferred into phase B; output DMAs are split by
    partition range across queues.
  - No blind PE warm-up block: real QKV matmuls + a few dummy matmuls warm
    the HAM clock-gate, and the phases keep the PE nearly saturated so it
    never re-throttles to half clock.
  - Head-major two-phase loop: heads {0,1} accumulate into o_a and finish
    before heads {2,3} start into o_b; o_a's normalization/projection
    overlap phase B, so only o_b's epilogue is a serial tail.
  - PV pairing: the two heads of a phase write disjoint 64-column groups
    of the PE array (tile_position (0,0)/(0,64)); adjacent emission makes
    them run concurrently => PV is ~1024 cycles per phase-jt.
  - S-tile triple buffering: the S/exp pipeline borrows the idle
    accumulator banks of the other phase (v_ps banks during phase A,
    o_a/y banks mid-phase-B) so S(jt+1) hides under exp(jt).
  - exp split across engines: ACT computes true exp for most tiles; the
    odd head of each phase on QUAD_A/QUAD_B jts is computed on the Vector
    engine as a fitted quadratic E' = c*(x+a)^2 (t = sqrt(c)*x+sqrt(c)*a
    from PSUM, then E' = t*t in bf16 at 2x mode). The constant term c*b of
    the fit c*((x+a)^2+b) ~ exp(x) is folded into the PV accumulation as a
    rank-1 correction matmul computed on-device from V, and softmax is
    invariant to the per-head scale c. End-to-end adds ~2e-3.
  - Token-axis l2norm scales fold into one per-(h,d) factor
    s = 10/(||q|| ||k||) applied to Q by an ACT Copy with per-partition
    scale; norms via ACT Square accumulation + a compressed DVE bit-hack
    rsqrt on the combined product.
  - S matmuls use block-diagonal K stationaries (one head's 32 rows live,
    rest zero); softmax denominators ride as `ones` rows of the [V|1] PV
    stationaries; normalization uses reciprocal_approx_fast + partition
    remap DMAs into a background-1.0 tile, junk rows killed by zero rows
    of the zero-padded per-head-pair w_out inputs.
"""

import os
import numpy as np
import ml_dtypes
from contextlib import ExitStack

import concourse.tile as tile
from concourse import bacc, mybir
from concourse.bass_utils import run_bass_kernel_spmd

FP32 = mybir.dt.float32
BF16 = mybir.dt.bfloat16

HW = 1024          # tokens per batch element (32*32)
C = 128            # channels
HEADS = 4
DH = 32            # dim per head
N_CORES = 8
NT = HW // 128     # 8 token tiles

# packed input layout (bf16 columns): [xt 1024 | wqkv 384 | woa 128 | wob 128]
INP_W = 1024 + 3 * C + C + C

# Quadratic exp fit: c*((x+a)^2 + b) ~ exp(x) on the logit range [-0.65, 0.55]
QA = 1.106669
QB = 0.949980
QC = 0.461088
SQC = QC ** 0.5          # folded into the DVE pass so E' = c*(x+a)^2
CB = QC * QB             # constant term, folded into PV via corr matmul

# jt values (per phase) whose ODD head S-tile is computed on the DVE
# quadratic instead of ACT exp.
QUAD_A = tuple(
    int(t) for t in os.environ.get("QUADA", "1,2,4,6,7").split(",") if t != ""
)
QUAD_B = tuple(
    int(t) for t in os.environ.get("QUADB", "2,4,6").split(",") if t != ""
)
N_DUM_PRE = int(os.environ.get("DUMPRE", "6"))
N_DUM_MID = int(os.environ.get("DUMMID", "7"))
DUM_ANCHOR = os.environ.get("DUMANCHOR", "1") == "1"
N_DUM_POST = int(os.environ.get("DUMPOST", "0"))
N_DUM_SEAM = int(os.environ.get("DUMSEAM", "0"))
# phase-B st-tile borrow positions (indices into the 16-tile stream) taken
# from the o_a/y banks once o_a's epilogue has read them; phase A borrows
# every 3rd tile from the (idle) v banks.
BORROW_B = tuple(
    int(t) for t in os.environ.get("BORROWB", "6,9,12").split(",") if t != ""
)


def build_kernel_body(ctx, tc, out_d, inp_d, bias_d):
    nc = tc.nc
    Exp = mybir.ActivationFunctionType.Exp
    Square = mybir.ActivationFunctionType.Square
    Copy = mybir.ActivationFunctionType.Copy
    mult = mybir.AluOpType.mult
    add = mybir.AluOpType.add
    shr = mybir.AluOpType.logical_shift_right

    const = ctx.enter_context(tc.tile_pool(name="const", bufs=1))
    sb = ctx.enter_context(tc.tile_pool(name="sb", bufs=1))
    tqp = ctx.enter_context(tc.tile_pool(name="tqp", bufs=2))
    # PSUM: stp rotates 2x 4KB/partition tiles (2 banks each);
    # ops (o_a -> phase-B st borrow -> y) and rps (v -> phase-A st borrow
    # -> o_b) 2 banks each. 4 + 2 + 2 = 8 banks.
    stp = ctx.enter_context(tc.tile_pool(name="stp", bufs=2, space="PSUM"))
    ops = ctx.enter_context(tc.tile_pool(name="ops", bufs=1, space="PSUM"))
    rps = ctx.enter_context(tc.tile_pool(name="rps", bufs=1, space="PSUM"))

    # ---- ACT table warm-up: touch Exp and Square immediately so the table
    # load overlaps the input DMAs instead of stalling the prologue.
    warm = const.tile([128, 1], FP32, tag="warm")
    nc.vector.memset(warm[:], 1.0)
    warm2 = const.tile([128, 1], FP32, tag="warm2")
    nc.scalar.activation(warm2[:], warm[:], Exp)
    nc.scalar.activation(warm2[:], warm[:], Square)

    # ---- packed input: ONE full-width DMA on sync. Measured: a single
    # [128, w] HWDGE DMA completes in ~3.6us end-to-end, while splitting it
    # across queues or partition ranges makes every in-flight DMA pile up
    # to 10-15us (packet-level round-robin + per-DMA completion receipt).
    inp = sb.tile([128, INP_W], BF16, tag="inp")
    nc.sync.dma_start(inp[:, :], inp_d[:, :])
    xtb = inp[:, 0:1024]
    wqb = inp[:, 1024:1024 + 3 * C]
    woa = inp[:, 1024 + 3 * C:1024 + 4 * C]
    wob = inp[:, 1024 + 4 * C:1024 + 5 * C]

    # ---- DVE/GPSIMD-built constants & backgrounds
    # wmm: 1.0s; dummy-matmul operand, ones-column for sum-of-V matmuls,
    # ones-row moving operand of corr matmuls. First so dummies start early.
    wmm = const.tile([128, 512], BF16, tag="wmm")
    nc.vector.memset(wmm[:], 1.0)
    # ktbd zero background: two memsets on DVE, two on GPSIMD
    ktbd = sb.tile([128, HEADS, 1024], BF16, tag="ktbd")
    nc.vector.memset(ktbd[:, 0, :], 0.0)
    nc.vector.memset(ktbd[:, 1, :], 0.0)
    nc.gpsimd.memset(ktbd[:, 2, :], 0.0)
    nc.gpsimd.memset(ktbd[:, 3, :], 0.0)
    # vb2[(j%128), t, h, 0:32] = V rows, [..., 32:64] = 1.0 (denominator)
    vb2 = sb.tile([128, NT, HEADS, 2 * DH], BF16, tag="vb2")
    nc.vector.memset(vb2[:, :, :, DH:2 * DH], 1.0)
    corr_sb = const.tile([1, 256], BF16, tag="corr_sb")
    nc.vector.memset(corr_sb[:], 0.0)

    bias = const.tile([128, NT, C], FP32, tag="bias")

    # PSUM persistent tiles. kt before qt so the sv tile (3rd stp
    # allocation) lands on kt's buffer (free early), not qt's (read late).
    kt_ps = stp.tile([128, 1024], FP32, tag="st", name="kt")
    qt_ps = stp.tile([128, 1024], FP32, tag="st", name="qt")
    o_a = ops.tile([128, 1024], FP32, tag="oacc", name="o_a")
    v_ps = rps.tile([128, 1024], FP32, tag="vacc", name="v_ps")

    # Prologue PE stream pinned into one dependency chain so the
    # scheduler's DMA-timing model cannot reorder it.
    pe_prev = [None]

    def pe_pin(bi):
        if pe_prev[0] is not None:
            tile.add_dep_helper(bi.ins, pe_prev[0].ins,
                                reason="pin prologue PE order")
        pe_prev[0] = bi

    def dummy_mm(n, target):
        for _ in range(n):
            pe_pin(nc.tensor.matmul(
                target[:, 0:512], lhsT=wmm[:, 0:128], rhs=wmm[:],
                start=True, stop=True, skip_group_check=True,
            ))

    # ---- pre-data dummies: PE activity while the input DMAs land, so the
    # HAM clock-gate ramp starts as early as possible.
    dummy_mm(N_DUM_PRE, o_a)

    # ---- Q^T then K^T then V
    for ih in range(2):
        pe_pin(nc.tensor.matmul(
            qt_ps[:, ih * 512:(ih + 1) * 512],
            lhsT=wqb[:, 0:C],
            rhs=xtb[:, ih * 512:(ih + 1) * 512],
            start=True, stop=True,
        ))
    for ih in range(2):
        pe_pin(nc.tensor.matmul(
            kt_ps[:, ih * 512:(ih + 1) * 512],
            lhsT=wqb[:, C:2 * C],
            rhs=xtb[:, ih * 512:(ih + 1) * 512],
            start=True, stop=True,
        ))
    for t in range(NT):
        pe_pin(nc.tensor.matmul(
            v_ps[:, t * 128:(t + 1) * 128],
            lhsT=xtb[:, t * 128:(t + 1) * 128],
            rhs=wqb[:, 2 * C:3 * C],
            start=True, stop=True,
        ))

    # ---- norm chain, spread across ACT and DVE ----
    # DVE: K^T to bf16 (source for the stripe DMAs) — emitted FIRST so the
    # scheduler runs it as soon as the K matmuls land, in parallel with the
    # ACT Squares, and it does not sit between ksq and the rsq chain.
    ktb = sb.tile([128, 1024], BF16, tag="ktb")
    nc.vector.tensor_copy(ktb[:], kt_ps[:])
    # ACT: Squares with free-axis accumulation -> nsq = [sum q^2, sum(.1k)^2]
    nsq = sb.tile([128, 2], FP32, tag="nsq")
    sq_scr = sb.tile([128, 1024], FP32, tag="sq_scr")
    sq1_i = nc.scalar.activation(sq_scr[:], qt_ps[:], Square,
                                 accum_out=nsq[:, 0:1])
    sq2_i = nc.scalar.activation(sq_scr[:], kt_ps[:], Square, scale=0.1,
                                 accum_out=nsq[:, 1:2])

    # combined scale = rsqrt(nq2 * 0.01*nk2) = 10/(||q|| ||k||) via the
    # fp32 bit-hack + 1 Newton step (compressed: one chain on the product).
    chain_is = []
    m0 = sb.tile([128, 1], FP32, tag="m0")
    chain_is.append(nc.vector.tensor_mul(m0[:], nsq[:, 0:1], nsq[:, 1:2]))
    mi = m0[:].bitcast(mybir.dt.int32)
    yi = sb.tile([128, 1], mybir.dt.int32, tag="yi")
    chain_is.append(nc.vector.tensor_scalar(yi[:], mi, 1, None, op0=shr))
    chain_is.append(nc.vector.tensor_scalar(yi[:], yi[:], -1, 0x5F3759DF,
                                            op0=mult, op1=add))
    y = yi[:].bitcast(FP32)
    t1 = sb.tile([128, 1], FP32, tag="t1")
    chain_is.append(nc.vector.tensor_mul(t1[:], y, y))
    chain_is.append(nc.vector.tensor_mul(t1[:], t1[:], m0[:]))
    chain_is.append(nc.vector.tensor_scalar(t1[:], t1[:], -0.5, 1.5,
                                            op0=mult, op1=add))
    rcomb = sb.tile([128, 1], FP32, tag="rcomb")
    rcomb_i = nc.vector.tensor_mul(rcomb[:], y, t1[:])
    chain_is.append(rcomb_i)

    # DVE: V scatter for the odd heads (the sum-of-V matmuls need them);
    # pinned behind the chain so it can't interleave into its sem gaps.
    cast_h13 = nc.vector.tensor_copy(
        vb2[:, :, 1::2, 0:DH],
        v_ps[:].rearrange("p (t h d) -> p t h d", t=NT, h=HEADS)[:, :, 1::2, :],
    )
    tile.add_dep_helper(cast_h13.ins, rcomb_i.ins, reason="after rsq chain")

    # block-diagonal K stripes via SBUF->SBUF DMA (h0 first: gates S(0,0))
    nc.sync.dma_start(ktbd[0:32, 0, :], ktb[0:32, :])
    nc.gpsimd.dma_start(ktbd[32:64, 1, :], ktb[32:64, :])
    nc.sync.dma_start(ktbd[64:96, 2, :], ktb[64:96, :])
    nc.gpsimd.dma_start(ktbd[96:128, 3, :], ktb[96:128, :])

    # mid-prologue dummies keep the PE busy through the norm chain; each is
    # tied to a norm-chain event so leftovers can never queue up in front
    # of the first real S matmuls (the PE pops ready work by priority).
    anchors = [sq1_i, sq1_i, sq1_i, sq2_i,
               chain_is[2], chain_is[4], chain_is[6]]
    for di in range(N_DUM_MID):
        bi = nc.tensor.matmul(
            o_a[:, 0:512], lhsT=wmm[:, 0:128], rhs=wmm[:],
            start=True, stop=True, skip_group_check=True,
        )
        pe_pin(bi)
        if DUM_ANCHOR:
            anchor = anchors[min(di, len(anchors) - 1)]
            tile.add_dep_helper(bi.ins, anchor.ins,
                                reason="trickle with chain")

    # ---- sum-of-V for the quadratic correction (odd head of each phase,
    # over that phase's QUAD tiles): sv[0, h*64+m] = sum_j vb2[j, jt, h, m]
    sv_ps = stp.tile([128, 1024], FP32, tag="st", name="sv")
    first_sv = True
    for h, quad in ((1, QUAD_A), (3, QUAD_B)):
        for jt in quad:
            pe_pin(nc.tensor.matmul(
                sv_ps[0:1, h * 64:(h + 1) * 64],
                lhsT=wmm[:, 0:1],
                rhs=vb2[:, jt, h, :],
                start=first_sv, stop=False,
                skip_group_check=True,
            ))
            first_sv = False
    if QUAD_A:
        nc.vector.tensor_scalar_mul(corr_sb[0:1, 64:128],
                                    sv_ps[0:1, 64:128], CB)
    if QUAD_B:
        nc.vector.tensor_scalar_mul(corr_sb[0:1, 192:256],
                                    sv_ps[0:1, 192:256], CB)

    # post-norm-chain dummies: the PE would otherwise idle for the rest of
    # the norm chain (rsq + qtb) and the HAM clock-gate would re-throttle
    # right as the S-pass starts.
    dummy_mm(N_DUM_POST, o_a)

    # Q scaled by the combined factor, on ACT, split in column halves so
    # S(0,0) ih0 can start after the first half.
    qtb = sb.tile([128, 1024], BF16, tag="qtb")
    nc.scalar.activation(qtb[:, 0:512], qt_ps[:, 0:512], Copy,
                         scale=rcomb[:, 0:1])
    nc.scalar.activation(qtb[:, 512:1024], qt_ps[:, 512:1024], Copy,
                         scale=rcomb[:, 0:1])

    # DVE: remaining V scatter (even heads) and the 1.0 backgrounds for
    # the normalization remaps — pinned behind the rsq chain so the
    # scheduler cannot interleave them into its semaphore gaps.
    cast_h02 = nc.vector.tensor_copy(
        vb2[:, :, 0::2, 0:DH],
        v_ps[:].rearrange("p (t h d) -> p t h d", t=NT, h=HEADS)[:, :, 0::2, :],
    )
    tile.add_dep_helper(cast_h02.ins, rcomb_i.ins, reason="after rsq chain")
    rash = sb.tile([128, 1024], FP32, tag="rash")
    ms1 = nc.vector.memset(rash[:], 1.0)
    tile.add_dep_helper(ms1.ins, rcomb_i.ins, reason="after rsq chain")
    rbsh = sb.tile([128, 1024], FP32, tag="rbsh")
    ms2 = nc.vector.memset(rbsh[:], 1.0)
    tile.add_dep_helper(ms2.ins, rcomb_i.ins, reason="after rsq chain")

    eb_a = sb.tile([128, NT, 2, 1024], BF16, tag="eb_a")
    eb_b = sb.tile([128, NT, 2, 1024], BF16, tag="eb_b")
    ra = sb.tile([128, 1024], FP32, tag="ra")
    rb = sb.tile([128, 1024], FP32, tag="rb")
    stack_a = sb.tile([128, 1024], BF16, tag="stack_a")
    stack_b = sb.tile([128, 1024], BF16, tag="stack_b")
    yout = sb.tile([128, NT, C], FP32, tag="yout")
    out_v = out_d.rearrange("(p t) c -> p t c", p=128)
    y_holder = [None]

    def emit_s(st, jt, h):
        for ih in range(2):
            nc.tensor.matmul(
                st[:, ih * 512:(ih + 1) * 512],
                lhsT=ktbd[:, h, jt * 128:(jt + 1) * 128],
                rhs=qtb[:, ih * 512:(ih + 1) * 512],
                start=True, stop=True,
            )

    def emit_quad(st, eb, jt):
        tq = tqp.tile([128, 1024], BF16, tag="tq")
        nc.vector.tensor_scalar(tq[:], st[:], SQC, SQC * QA,
                                op0=mult, op1=add)
        nc.vector.tensor_mul(eb[:, jt, 1, :], tq[:], tq[:])

    def emit_corr(o, hp):
        # opens the accumulation group: start=True clears each bank's
        # has_written, then writes the rank-1 quadratic correction.
        for ih in range(2):
            pe_pin(nc.tensor.matmul(
                o[:, ih * 512:(ih + 1) * 512],
                lhsT=corr_sb[0:1, hp * 128:(hp + 1) * 128],
                rhs=wmm[0:1, :],
                start=True, stop=False,
                skip_group_check=True,
            ))

    def emit_pv(o, eb, hp, jt, ih_order=(0, 1)):
        # two heads in disjoint PE column groups, emitted adjacently so
        # the hardware runs them concurrently.
        for ih in ih_order:
            for hh in range(2):
                nc.tensor.matmul(
                    o[64 * hh:64 * hh + 64, ih * 512:(ih + 1) * 512],
                    lhsT=vb2[:, jt, 2 * hp + hh, :],
                    rhs=eb[:, jt, hh, ih * 512:(ih + 1) * 512],
                    start=False, stop=(jt == NT - 1),
                    tile_position=(0, 64 * hh),
                    skip_group_check=True,
                )

    def emit_phase(hp, o, eb, quad, st_alloc):
        for jt in range(NT):
            st0 = st_alloc(2 * jt)
            emit_s(st0, jt, 2 * hp)
            st1 = st_alloc(2 * jt + 1)
            emit_s(st1, jt, 2 * hp + 1)
            nc.scalar.activation(eb[:, jt, 0, :], st0[:], Exp)
            if jt in quad:
                emit_quad(st1, eb, jt)
            else:
                nc.scalar.activation(eb[:, jt, 1, :], st1[:], Exp)
            if jt == 0:
                # corr opens the accumulation group; emitted after jt0's S
                # tiles so it cannot delay the phase's pipeline restart
                # (it is only needed before the first PV, at jt1).
                emit_corr(o, hp)
            if jt > 0:
                emit_pv(o, eb, hp, jt - 1)
        # the final PV emits column-half 1 first so the epilogue's
        # second-half recip/remap chain starts as early as the first's.
        emit_pv(o, eb, hp, NT - 1, ih_order=(1, 0) if hp == 1 else (0, 1))

    def st_alloc_a(i):
        # every 3rd S-tile borrows the v_ps banks (idle during phase A)
        if i % 3 == 2:
            return rps.tile([128, 1024], FP32, tag="vacc", name=f"stA_{i}")
        return stp.tile([128, 1024], FP32, tag="st", name=f"stA_{i}")

    def st_alloc_b(i):
        if i in BORROW_B:
            return ops.tile([128, 1024], FP32, tag="oacc", name=f"stB_{i}")
        return stp.tile([128, 1024], FP32, tag="st", name=f"stB_{i}")

    def emit_remaps(rr, rsh, cs, flip):
        # partition remap of the denominator reciprocals on the two free
        # DMA queues (SBUF->SBUF, cheap).
        q0 = nc.sync if not flip else nc.gpsimd
        q1 = nc.gpsimd if not flip else nc.sync
        q0.dma_start(rsh[0:32, cs], rr[32:64, cs])
        q1.dma_start(rsh[64:96, cs], rr[96:128, cs])

    def emit_epilogue(hp, o, rr, rsh, stack, w_t, proj=True):
        # column halves; all recips + remap DMAs are issued up-front so the
        # two halves' remap completion receipts (~2.4us each) overlap
        # instead of chaining serially into the tail.
        w = 512
        outq = (nc.gpsimd, nc.sync)
        for ch in range(2):
            cs = slice(ch * w, (ch + 1) * w)
            nc.vector.reciprocal_approx_fast(rr[:, cs], o[:, cs])
            emit_remaps(rr, rsh, cs, flip=(ch % 2 == 1))
        for ch in range(2):
            cs = slice(ch * w, (ch + 1) * w)
            nc.vector.tensor_mul(stack[:, cs], o[:, cs], rsh[:, cs])
            if proj:
                emit_proj(hp, stack, w_t, ch, w // 128, outq[ch])

    def emit_proj(hp, stack, w_t, ch, nit, outq=None):
        if y_holder[0] is None:
            y_holder[0] = ops.tile([128, 1024], FP32, tag="oacc",
                                   name="y_ps")
        y_ps = y_holder[0]
        for it in range(ch * nit, ch * nit + nit):
            nc.tensor.matmul(
                y_ps[:, it * 128:(it + 1) * 128],
                lhsT=stack[:, it * 128:(it + 1) * 128],
                rhs=w_t[:],
                start=(hp == 0 and it % 4 == 0),
                stop=(hp == 1 and it % 4 == 3),
                skip_group_check=True,
            )
        if hp == 1:
            y_v = y_ps[:].rearrange("p (t c) -> p t c", t=NT)
            t0, t1 = ch * nit, ch * nit + nit
            nc.vector.tensor_add(yout[:, t0:t1, :], y_v[:, t0:t1, :],
                                 bias[:, t0:t1, :])
            outq.dma_start(out_v[:, t0:t1, :], yout[:, t0:t1, :])

    emit_phase(0, o_a, eb_a, QUAD_A, st_alloc_a)
    # phase A epilogue: recip/remap/mul overlap phase B; the projection is
    # emitted after phase B's borrowed tiles so the y banks are free.
    emit_epilogue(0, o_a, ra, rash, stack_a, woa, proj=False)
    o_b = rps.tile([128, 1024], FP32, tag="vacc", name="o_b")
    # seam dummies: PE filler while phase B's pipeline refills
    dummy_mm(N_DUM_SEAM, o_b)
    # deferred (zero) bias load: single sync DMA during phase B, pinned
    # behind the seam so the scheduler cannot float it into the prologue
    # where it would interfere with the critical input DMA.
    bias_i = nc.sync.dma_start(bias[:], bias_d[:])
    tile.add_dep_helper(bias_i.ins, pe_prev[0].ins,
                        reason="defer bias load past the seam")
    emit_phase(1, o_b, eb_b, QUAD_B, st_alloc_b)
    emit_proj(0, stack_a, woa, 0, 4)
    emit_proj(0, stack_a, woa, 1, 4)
    emit_epilogue(1, o_b, rb, rbsh, stack_b, wob, proj=True)


def build_nc():
    nc = bacc.Bacc("TRN2", target_bir_lowering=False, debug=False,
                   num_devices=N_CORES)
    inp_d = nc.dram_tensor("inp", [128, INP_W], BF16,
                           kind="ExternalInput").ap()
    bias_d = nc.dram_tensor("bias", [128, NT, C], FP32,
                            kind="ExternalInput").ap()
    out_d = nc.dram_tensor("out", [HW, C], FP32, kind="ExternalOutput").ap()
    with tile.TileContext(nc) as tc:
        with ExitStack() as ctx:
            build_kernel_body(ctx, tc, out_d, inp_d, bias_d)
    nc.compile()
    return nc


_CACHED_NC = None


def get_nc():
    global _CACHED_NC
    if _CACHED_NC is None:
        _CACHED_NC = build_nc()
    return _CACHED_NC


def make_in_maps(x, w_qkv, w_out, b_out):
    x = np.ascontiguousarray(np.asarray(x, dtype=np.float32)).reshape(N_CORES, HW, C)
    # [c, (t, p)] with column t*128+p = token 8p+t, bf16
    xt = np.ascontiguousarray(
        x.reshape(N_CORES, 128, NT, C).transpose(0, 3, 2, 1).reshape(N_CORES, C, HW)
    ).astype(ml_dtypes.bfloat16)
    w_qkv_bf = np.asarray(w_qkv, dtype=np.float32).astype(ml_dtypes.bfloat16)
    w_out = np.asarray(w_out, dtype=np.float32)
    b_out = np.asarray(b_out, dtype=np.float32).reshape(C)

    # woa: rows [w_out[0:32]; 0; w_out[32:64]; 0]  (heads 0, 1)
    # wob: rows [w_out[64:96]; 0; w_out[96:128]; 0]  (heads 2, 3)
    woa = np.zeros((128, C), dtype=np.float32)
    wob = np.zeros((128, C), dtype=np.float32)
    woa[0:32] = w_out[0:32]
    woa[64:96] = w_out[32:64]
    wob[0:32] = w_out[64:96]
    wob[64:96] = w_out[96:128]
    woa = woa.astype(ml_dtypes.bfloat16)
    wob = wob.astype(ml_dtypes.bfloat16)
    bias = np.ascontiguousarray(
        np.broadcast_to(b_out[None, None, :], (128, NT, C)).astype(np.float32))
    inp = np.concatenate(
        [xt, np.broadcast_to(w_qkv_bf[None], (N_CORES, C, 3 * C)),
         np.broadcast_to(woa[None], (N_CORES, 128, C)),
         np.broadcast_to(wob[None], (N_CORES, 128, C))], axis=2)
    inp = np.ascontiguousarray(inp).astype(ml_dtypes.bfloat16)
    return [
        {"inp": inp[i], "bias": bias}
        for i in range(N_CORES)
    ]


def kernel(x, w_qkv, w_out, b_out, _trace=False, _trace_kwargs=None):
    nc = get_nc()
    in_maps = make_in_maps(x, w_qkv, w_out, b_out)
    res = run_bass_kernel_spmd(
        nc, in_maps, core_ids=list(range(N_CORES)),
        trace=_trace, **(_trace_kwargs or {}),
    )
    out = np.stack([np.asarray(res.results[i]["out"]) for i in range(N_CORES)])
    out = out.reshape(8, 32, 32, 128).astype(np.float32)
    if _trace:
        kernel.last_result = res
    return out


# revision 53
# speedup vs baseline: 1.0362x; 1.0013x over previous
"""Trainium2 Bass kernel for nn_Attention_61168924229643.

4-head attention over 1024 tokens, dim_head=32, with the reference's quirks:
  - l2norm over the TOKEN axis (axis=1 of (B, HW, h, d)),
  - `attn - attn.argmax(-1)` before softmax: a per-row constant shift that
    cancels exactly inside jax.nn.softmax. Logits are bounded (|S| < 0.6),
    so a raw exp/sum softmax reproduces the reference to ~2e-5.

Sharding: B=8 batch elements -> one NeuronCore each, no collectives.

Layout: tokens on the SBUF free axis, channels on partitions ("transposed").
x arrives host-transposed/bf16-cast; attention is permutation-equivariant
over tokens and a permuted token order (token 8p+t <-> column t*128+p) makes
both the input and output DMAs contiguous per partition.

Performance structure (v4):
  - DMA cost here is ~42ns per partition-row packet, so [128, n] transfers
    are ~5.4us regardless of width. All critical inputs (xt, w_qkv, w_out
    tiles) are packed into ONE bf16 DRAM tensor and loaded as 3
    partition-range slices on the 3 DMA-capable queues (~2.5us landing).
    The ktbd zero background is DVE/GPSIMD memsets instead of DMA; the
    (zero) bias load is deferred into phase B; output DMAs are split by
    partition range across queues.
  - No blind PE warm-up block: real QKV matmuls + a few dummy matmuls warm
    the HAM clock-gate, and the phases keep the PE nearly saturated so it
    never re-throttles to half clock.
  - Head-major two-phase loop: heads {0,1} accumulate into o_a and finish
    before heads {2,3} start into o_b; o_a's normalization/projection
    overlap phase B, so only o_b's epilogue is a serial tail.
  - PV pairing: the two heads of a phase write disjoint 64-column groups
    of the PE array (tile_position (0,0)/(0,64)); adjacent emission makes
    them run concurrently => PV is ~1024 cycles per phase-jt.
  - S-tile triple buffering: the S/exp pipeline borrows the idle
    accumulator banks of the other phase (v_ps banks during phase A,
    o_a/y banks mid-phase-B) so S(jt+1) hides under exp(jt).
  - exp split across engines: ACT computes true exp for most tiles; the
    odd head of each phase on QUAD_A/QUAD_B jts is computed on the Vector
    engine as a fitted quadratic E' = c*(x+a)^2 (t = sqrt(c)*x+sqrt(c)*a
    from PSUM, then E' = t*t in bf16 at 2x mode). The constant term c*b of
    the fit c*((x+a)^2+b) ~ exp(x) is folded into the PV accumulation as a
    rank-1 correction matmul computed on-device from V, and softmax is
    invariant to the per-head scale c. End-to-end adds ~2e-3.
  - Token-axis l2norm scales fold into one per-(h,d) factor
    s = 10/(||q|| ||k||) applied to Q by an ACT Copy with per-partition
    scale; norms via ACT Square accumulation + a compressed DVE bit-hack
    rsqrt on the combined product.
  - S matmuls use block-diagonal K stationaries (one head's 32 rows live,
    rest zero); softmax denominators ride as `ones` rows of the [V|1] PV
    stationaries; normalization uses reciprocal_approx_fast + partition
    remap DMAs into a background-1.0 tile, junk rows killed by zero rows
    of the zero-padded per-head-pair w_out inputs.
"""

import os
import numpy as np
import ml_dtypes
from contextlib import ExitStack

import concourse.tile as tile
from concourse import bacc, mybir
from concourse.bass_utils import run_bass_kernel_spmd

FP32 = mybir.dt.float32
BF16 = mybir.dt.bfloat16

HW = 1024          # tokens per batch element (32*32)
C = 128            # channels
HEADS = 4
DH = 32            # dim per head
N_CORES = 8
NT = HW // 128     # 8 token tiles

# packed input layout (bf16 columns): [xt 1024 | wqkv 384 | woa 128 | wob 128]
INP_W = 1024 + 3 * C + C + C

# Quadratic exp fit: c*((x+a)^2 + b) ~ exp(x) on the logit range [-0.65, 0.55]
QA = 1.106669
QB = 0.949980
QC = 0.461088
SQC = QC ** 0.5          # folded into the DVE pass so E' = c*(x+a)^2
CB = QC * QB             # constant term, folded into PV via corr matmul

# jt values (per phase) whose ODD head S-tile is computed on the DVE
# quadratic instead of ACT exp.
QUAD_A = tuple(
    int(t) for t in os.environ.get("QUADA", "1,2,4,6,7").split(",") if t != ""
)
QUAD_B = tuple(
    int(t) for t in os.environ.get("QUADB", "2,4,6").split(",") if t != ""
)
N_DUM_PRE = int(os.environ.get("DUMPRE", "5"))
N_DUM_MID = int(os.environ.get("DUMMID", "7"))
DUM_ANCHOR = os.environ.get("DUMANCHOR", "1") == "1"
N_DUM_POST = int(os.environ.get("DUMPOST", "0"))
N_DUM_SEAM = int(os.environ.get("DUMSEAM", "0"))
# phase-B st-tile borrow positions (indices into the 16-tile stream) taken
# from the o_a/y banks once o_a's epilogue has read them; phase A borrows
# every 3rd tile from the (idle) v banks.
BORROW_B = tuple(
    int(t) for t in os.environ.get("BORROWB", "6,9,12").split(",") if t != ""
)


def build_kernel_body(ctx, tc, out_d, inp_d, bias_d):
    nc = tc.nc
    Exp = mybir.ActivationFunctionType.Exp
    Square = mybir.ActivationFunctionType.Square
    Copy = mybir.ActivationFunctionType.Copy
    mult = mybir.AluOpType.mult
    add = mybir.AluOpType.add
    shr = mybir.AluOpType.logical_shift_right

    const = ctx.enter_context(tc.tile_pool(name="const", bufs=1))
    sb = ctx.enter_context(tc.tile_pool(name="sb", bufs=1))
    tqp = ctx.enter_context(tc.tile_pool(name="tqp", bufs=2))
    # PSUM: stp rotates 2x 4KB/partition tiles (2 banks each);
    # ops (o_a -> phase-B st borrow -> y) and rps (v -> phase-A st borrow
    # -> o_b) 2 banks each. 4 + 2 + 2 = 8 banks.
    stp = ctx.enter_context(tc.tile_pool(name="stp", bufs=2, space="PSUM"))
    ops = ctx.enter_context(tc.tile_pool(name="ops", bufs=1, space="PSUM"))
    rps = ctx.enter_context(tc.tile_pool(name="rps", bufs=1, space="PSUM"))

    # ---- ACT table warm-up: touch Exp and Square immediately so the table
    # load overlaps the input DMAs instead of stalling the prologue.
    warm = const.tile([128, 1], FP32, tag="warm")
    nc.vector.memset(warm[:], 1.0)
    warm2 = const.tile([128, 1], FP32, tag="warm2")
    nc.scalar.activation(warm2[:], warm[:], Exp)
    nc.scalar.activation(warm2[:], warm[:], Square)

    # ---- packed input: ONE full-width DMA on sync. Measured: a single
    # [128, w] HWDGE DMA completes in ~3.6us end-to-end, while splitting it
    # across queues or partition ranges makes every in-flight DMA pile up
    # to 10-15us (packet-level round-robin + per-DMA completion receipt).
    inp = sb.tile([128, INP_W], BF16, tag="inp")
    nc.sync.dma_start(inp[:, :], inp_d[:, :])
    xtb = inp[:, 0:1024]
    wqb = inp[:, 1024:1024 + 3 * C]
    woa = inp[:, 1024 + 3 * C:1024 + 4 * C]
    wob = inp[:, 1024 + 4 * C:1024 + 5 * C]

    # ---- DVE/GPSIMD-built constants & backgrounds
    # wmm: 1.0s; dummy-matmul operand, ones-column for sum-of-V matmuls,
    # ones-row moving operand of corr matmuls. First so dummies start early.
    wmm = const.tile([128, 512], BF16, tag="wmm")
    nc.vector.memset(wmm[:], 1.0)
    # ktbd zero background: two memsets on DVE, two on GPSIMD
    ktbd = sb.tile([128, HEADS, 1024], BF16, tag="ktbd")
    nc.vector.memset(ktbd[:, 0, :], 0.0)
    nc.vector.memset(ktbd[:, 1, :], 0.0)
    nc.gpsimd.memset(ktbd[:, 2, :], 0.0)
    nc.gpsimd.memset(ktbd[:, 3, :], 0.0)
    # vb2[(j%128), t, h, 0:32] = V rows, [..., 32:64] = 1.0 (denominator)
    vb2 = sb.tile([128, NT, HEADS, 2 * DH], BF16, tag="vb2")
    nc.vector.memset(vb2[:, :, :, DH:2 * DH], 1.0)
    corr_sb = const.tile([1, 256], BF16, tag="corr_sb")
    nc.vector.memset(corr_sb[:], 0.0)

    bias = const.tile([128, NT, C], FP32, tag="bias")

    # PSUM persistent tiles. kt before qt so the sv tile (3rd stp
    # allocation) lands on kt's buffer (free early), not qt's (read late).
    kt_ps = stp.tile([128, 1024], FP32, tag="st", name="kt")
    qt_ps = stp.tile([128, 1024], FP32, tag="st", name="qt")
    o_a = ops.tile([128, 1024], FP32, tag="oacc", name="o_a")
    v_ps = rps.tile([128, 1024], FP32, tag="vacc", name="v_ps")

    # Prologue PE stream pinned into one dependency chain so the
    # scheduler's DMA-timing model cannot reorder it.
    pe_prev = [None]

    def pe_pin(bi):
        if pe_prev[0] is not None:
            tile.add_dep_helper(bi.ins, pe_prev[0].ins,
                                reason="pin prologue PE order")
        pe_prev[0] = bi

    def dummy_mm(n, target):
        for _ in range(n):
            pe_pin(nc.tensor.matmul(
                target[:, 0:512], lhsT=wmm[:, 0:128], rhs=wmm[:],
                start=True, stop=True, skip_group_check=True,
            ))

    # ---- pre-data dummies: PE activity while the input DMAs land, so the
    # HAM clock-gate ramp starts as early as possible.
    dummy_mm(N_DUM_PRE, o_a)

    # ---- Q^T then K^T then V
    for ih in range(2):
        pe_pin(nc.tensor.matmul(
            qt_ps[:, ih * 512:(ih + 1) * 512],
            lhsT=wqb[:, 0:C],
            rhs=xtb[:, ih * 512:(ih + 1) * 512],
            start=True, stop=True,
        ))
    for ih in range(2):
        pe_pin(nc.tensor.matmul(
            kt_ps[:, ih * 512:(ih + 1) * 512],
            lhsT=wqb[:, C:2 * C],
            rhs=xtb[:, ih * 512:(ih + 1) * 512],
            start=True, stop=True,
        ))
    for t in range(NT):
        pe_pin(nc.tensor.matmul(
            v_ps[:, t * 128:(t + 1) * 128],
            lhsT=xtb[:, t * 128:(t + 1) * 128],
            rhs=wqb[:, 2 * C:3 * C],
            start=True, stop=True,
        ))

    # ---- norm chain, spread across ACT and DVE ----
    # DVE: K^T to bf16 (source for the stripe DMAs) — emitted FIRST so the
    # scheduler runs it as soon as the K matmuls land, in parallel with the
    # ACT Squares, and it does not sit between ksq and the rsq chain.
    ktb = sb.tile([128, 1024], BF16, tag="ktb")
    nc.vector.tensor_copy(ktb[:], kt_ps[:])
    # ACT: Squares with free-axis accumulation -> nsq = [sum q^2, sum(.1k)^2]
    nsq = sb.tile([128, 2], FP32, tag="nsq")
    sq_scr = sb.tile([128, 1024], FP32, tag="sq_scr")
    sq1_i = nc.scalar.activation(sq_scr[:], qt_ps[:], Square,
                                 accum_out=nsq[:, 0:1])
    sq2_i = nc.scalar.activation(sq_scr[:], kt_ps[:], Square, scale=0.1,
                                 accum_out=nsq[:, 1:2])

    # combined scale = rsqrt(nq2 * 0.01*nk2) = 10/(||q|| ||k||) via the
    # fp32 bit-hack + 1 Newton step (compressed: one chain on the product).
    chain_is = []
    m0 = sb.tile([128, 1], FP32, tag="m0")
    chain_is.append(nc.vector.tensor_mul(m0[:], nsq[:, 0:1], nsq[:, 1:2]))
    mi = m0[:].bitcast(mybir.dt.int32)
    yi = sb.tile([128, 1], mybir.dt.int32, tag="yi")
    chain_is.append(nc.vector.tensor_scalar(yi[:], mi, 1, None, op0=shr))
    chain_is.append(nc.vector.tensor_scalar(yi[:], yi[:], -1, 0x5F3759DF,
                                            op0=mult, op1=add))
    y = yi[:].bitcast(FP32)
    t1 = sb.tile([128, 1], FP32, tag="t1")
    chain_is.append(nc.vector.tensor_mul(t1[:], y, y))
    chain_is.append(nc.vector.tensor_mul(t1[:], t1[:], m0[:]))
    chain_is.append(nc.vector.tensor_scalar(t1[:], t1[:], -0.5, 1.5,
                                            op0=mult, op1=add))
    rcomb = sb.tile([128, 1], FP32, tag="rcomb")
    rcomb_i = nc.vector.tensor_mul(rcomb[:], y, t1[:])
    chain_is.append(rcomb_i)

    # DVE: V scatter for the odd heads (the sum-of-V matmuls need them);
    # pinned behind the chain so it can't interleave into its sem gaps.
    cast_h13 = nc.vector.tensor_copy(
        vb2[:, :, 1::2, 0:DH],
        v_ps[:].rearrange("p (t h d) -> p t h d", t=NT, h=HEADS)[:, :, 1::2, :],
    )
    tile.add_dep_helper(cast_h13.ins, rcomb_i.ins, reason="after rsq chain")

    # block-diagonal K stripes via SBUF->SBUF DMA (h0 first: gates S(0,0))
    nc.sync.dma_start(ktbd[0:32, 0, :], ktb[0:32, :])
    nc.gpsimd.dma_start(ktbd[32:64, 1, :], ktb[32:64, :])
    nc.sync.dma_start(ktbd[64:96, 2, :], ktb[64:96, :])
    nc.gpsimd.dma_start(ktbd[96:128, 3, :], ktb[96:128, :])

    # mid-prologue dummies keep the PE busy through the norm chain; each is
    # tied to a norm-chain event so leftovers can never queue up in front
    # of the first real S matmuls (the PE pops ready work by priority).
    anchors = [sq1_i, sq1_i, sq1_i, sq2_i,
               chain_is[2], chain_is[4], chain_is[6]]
    for di in range(N_DUM_MID):
        bi = nc.tensor.matmul(
            o_a[:, 0:512], lhsT=wmm[:, 0:128], rhs=wmm[:],
            start=True, stop=True, skip_group_check=True,
        )
        pe_pin(bi)
        if DUM_ANCHOR:
            anchor = anchors[min(di, len(anchors) - 1)]
            tile.add_dep_helper(bi.ins, anchor.ins,
                                reason="trickle with chain")

    # ---- sum-of-V for the quadratic correction (odd head of each phase,
    # over that phase's QUAD tiles): sv[0, h*64+m] = sum_j vb2[j, jt, h, m]
    sv_ps = stp.tile([128, 1024], FP32, tag="st", name="sv")
    first_sv = True
    for h, quad in ((1, QUAD_A), (3, QUAD_B)):
        for jt in quad:
            pe_pin(nc.tensor.matmul(
                sv_ps[0:1, h * 64:(h + 1) * 64],
                lhsT=wmm[:, 0:1],
                rhs=vb2[:, jt, h, :],
                start=first_sv, stop=False,
                skip_group_check=True,
            ))
            first_sv = False
    if QUAD_A:
        nc.vector.tensor_scalar_mul(corr_sb[0:1, 64:128],
                                    sv_ps[0:1, 64:128], CB)
    if QUAD_B:
        nc.vector.tensor_scalar_mul(corr_sb[0:1, 192:256],
                                    sv_ps[0:1, 192:256], CB)

    # post-norm-chain dummies: the PE would otherwise idle for the rest of
    # the norm chain (rsq + qtb) and the HAM clock-gate would re-throttle
    # right as the S-pass starts.
    dummy_mm(N_DUM_POST, o_a)

    # Q scaled by the combined factor, on ACT, split in column halves so
    # S(0,0) ih0 can start after the first half.
    qtb = sb.tile([128, 1024], BF16, tag="qtb")
    nc.scalar.activation(qtb[:, 0:512], qt_ps[:, 0:512], Copy,
                         scale=rcomb[:, 0:1])
    nc.scalar.activation(qtb[:, 512:1024], qt_ps[:, 512:1024], Copy,
                         scale=rcomb[:, 0:1])

    # DVE: remaining V scatter (even heads) and the 1.0 backgrounds for
    # the normalization remaps — pinned behind the rsq chain so the
    # scheduler cannot interleave them into its semaphore gaps.
    cast_h02 = nc.vector.tensor_copy(
        vb2[:, :, 0::2, 0:DH],
        v_ps[:].rearrange("p (t h d) -> p t h d", t=NT, h=HEADS)[:, :, 0::2, :],
    )
    tile.add_dep_helper(cast_h02.ins, rcomb_i.ins, reason="after rsq chain")
    rash = sb.tile([128, 1024], FP32, tag="rash")
    ms1 = nc.vector.memset(rash[:], 1.0)
    tile.add_dep_helper(ms1.ins, rcomb_i.ins, reason="after rsq chain")
    rbsh = sb.tile([128, 1024], FP32, tag="rbsh")
    ms2 = nc.vector.memset(rbsh[:], 1.0)
    tile.add_dep_helper(ms2.ins, rcomb_i.ins, reason="after rsq chain")

    eb_a = sb.tile([128, NT, 2, 1024], BF16, tag="eb_a")
    eb_b = sb.tile([128, NT, 2, 1024], BF16, tag="eb_b")
    ra = sb.tile([128, 1024], FP32, tag="ra")
    rb = sb.tile([128, 1024], FP32, tag="rb")
    stack_a = sb.tile([128, 1024], BF16, tag="stack_a")
    stack_b = sb.tile([128, 1024], BF16, tag="stack_b")
    yout = sb.tile([128, NT, C], FP32, tag="yout")
    out_v = out_d.rearrange("(p t) c -> p t c", p=128)
    y_holder = [None]

    def emit_s(st, jt, h):
        for ih in range(2):
            nc.tensor.matmul(
                st[:, ih * 512:(ih + 1) * 512],
                lhsT=ktbd[:, h, jt * 128:(jt + 1) * 128],
                rhs=qtb[:, ih * 512:(ih + 1) * 512],
                start=True, stop=True,
            )

    def emit_quad(st, eb, jt):
        tq = tqp.tile([128, 1024], BF16, tag="tq")
        nc.vector.tensor_scalar(tq[:], st[:], SQC, SQC * QA,
                                op0=mult, op1=add)
        nc.vector.tensor_mul(eb[:, jt, 1, :], tq[:], tq[:])

    def emit_corr(o, hp):
        # opens the accumulation group: start=True clears each bank's
        # has_written, then writes the rank-1 quadratic correction.
        for ih in range(2):
            pe_pin(nc.tensor.matmul(
                o[:, ih * 512:(ih + 1) * 512],
                lhsT=corr_sb[0:1, hp * 128:(hp + 1) * 128],
                rhs=wmm[0:1, :],
                start=True, stop=False,
                skip_group_check=True,
            ))

    def emit_pv(o, eb, hp, jt, ih_order=(0, 1)):
        # two heads in disjoint PE column groups, emitted adjacently so
        # the hardware runs them concurrently.
        for ih in ih_order:
            for hh in range(2):
                nc.tensor.matmul(
                    o[64 * hh:64 * hh + 64, ih * 512:(ih + 1) * 512],
                    lhsT=vb2[:, jt, 2 * hp + hh, :],
                    rhs=eb[:, jt, hh, ih * 512:(ih + 1) * 512],
                    start=False, stop=(jt == NT - 1),
                    tile_position=(0, 64 * hh),
                    skip_group_check=True,
                )

    def emit_phase(hp, o, eb, quad, st_alloc):
        for jt in range(NT):
            st0 = st_alloc(2 * jt)
            emit_s(st0, jt, 2 * hp)
            st1 = st_alloc(2 * jt + 1)
            emit_s(st1, jt, 2 * hp + 1)
            nc.scalar.activation(eb[:, jt, 0, :], st0[:], Exp)
            if jt in quad:
                emit_quad(st1, eb, jt)
            else:
                nc.scalar.activation(eb[:, jt, 1, :], st1[:], Exp)
            if jt == 0:
                # corr opens the accumulation group; emitted after jt0's S
                # tiles so it cannot delay the phase's pipeline restart
                # (it is only needed before the first PV, at jt1).
                emit_corr(o, hp)
            if jt > 0:
                emit_pv(o, eb, hp, jt - 1)
        # the final PV emits column-half 1 first so the epilogue's
        # second-half recip/remap chain starts as early as the first's.
        emit_pv(o, eb, hp, NT - 1, ih_order=(1, 0) if hp == 1 else (0, 1))

    def st_alloc_a(i):
        # every 3rd S-tile borrows the v_ps banks (idle during phase A)
        if i % 3 == 2:
            return rps.tile([128, 1024], FP32, tag="vacc", name=f"stA_{i}")
        return stp.tile([128, 1024], FP32, tag="st", name=f"stA_{i}")

    def st_alloc_b(i):
        if i in BORROW_B:
            return ops.tile([128, 1024], FP32, tag="oacc", name=f"stB_{i}")
        return stp.tile([128, 1024], FP32, tag="st", name=f"stB_{i}")

    def emit_remaps(rr, rsh, cs, flip):
        # partition remap of the denominator reciprocals on the two free
        # DMA queues (SBUF->SBUF, cheap).
        q0 = nc.sync if not flip else nc.gpsimd
        q1 = nc.gpsimd if not flip else nc.sync
        q0.dma_start(rsh[0:32, cs], rr[32:64, cs])
        q1.dma_start(rsh[64:96, cs], rr[96:128, cs])

    def emit_epilogue(hp, o, rr, rsh, stack, w_t, proj=True):
        # column halves; all recips + remap DMAs are issued up-front so the
        # two halves' remap completion receipts (~2.4us each) overlap
        # instead of chaining serially into the tail.
        w = 512
        outq = (nc.gpsimd, nc.sync)
        for ch in range(2):
            cs = slice(ch * w, (ch + 1) * w)
            nc.vector.reciprocal_approx_fast(rr[:, cs], o[:, cs])
            emit_remaps(rr, rsh, cs, flip=(ch % 2 == 1))
        for ch in range(2):
            cs = slice(ch * w, (ch + 1) * w)
            nc.vector.tensor_mul(stack[:, cs], o[:, cs], rsh[:, cs])
            if proj:
                emit_proj(hp, stack, w_t, ch, w // 128, outq[ch])

    def emit_proj(hp, stack, w_t, ch, nit, outq=None):
        if y_holder[0] is None:
            y_holder[0] = ops.tile([128, 1024], FP32, tag="oacc",
                                   name="y_ps")
        y_ps = y_holder[0]
        for it in range(ch * nit, ch * nit + nit):
            nc.tensor.matmul(
                y_ps[:, it * 128:(it + 1) * 128],
                lhsT=stack[:, it * 128:(it + 1) * 128],
                rhs=w_t[:],
                start=(hp == 0 and it % 4 == 0),
                stop=(hp == 1 and it % 4 == 3),
                skip_group_check=True,
            )
        if hp == 1:
            y_v = y_ps[:].rearrange("p (t c) -> p t c", t=NT)
            t0, t1 = ch * nit, ch * nit + nit
            nc.vector.tensor_add(yout[:, t0:t1, :], y_v[:, t0:t1, :],
                                 bias[:, t0:t1, :])
            outq.dma_start(out_v[:, t0:t1, :], yout[:, t0:t1, :])

    emit_phase(0, o_a, eb_a, QUAD_A, st_alloc_a)
    # phase A epilogue: recip/remap/mul overlap phase B; the projection is
    # emitted after phase B's borrowed tiles so the y banks are free.
    emit_epilogue(0, o_a, ra, rash, stack_a, woa, proj=False)
    o_b = rps.tile([128, 1024], FP32, tag="vacc", name="o_b")
    # seam dummies: PE filler while phase B's pipeline refills
    dummy_mm(N_DUM_SEAM, o_b)
    # deferred (zero) bias load: single sync DMA during phase B, pinned
    # behind the seam so the scheduler cannot float it into the prologue
    # where it would interfere with the critical input DMA.
    bias_i = nc.sync.dma_start(bias[:], bias_d[:])
    tile.add_dep_helper(bias_i.ins, pe_prev[0].ins,
                        reason="defer bias load past the seam")
    emit_phase(1, o_b, eb_b, QUAD_B, st_alloc_b)
    emit_proj(0, stack_a, woa, 0, 4)
    emit_proj(0, stack_a, woa, 1, 4)
    emit_epilogue(1, o_b, rb, rbsh, stack_b, wob, proj=True)


def build_nc():
    nc = bacc.Bacc("TRN2", target_bir_lowering=False, debug=False,
                   num_devices=N_CORES)
    inp_d = nc.dram_tensor("inp", [128, INP_W], BF16,
                           kind="ExternalInput").ap()
    bias_d = nc.dram_tensor("bias", [128, NT, C], FP32,
                            kind="ExternalInput").ap()
    out_d = nc.dram_tensor("out", [HW, C], FP32, kind="ExternalOutput").ap()
    with tile.TileContext(nc) as tc:
        with ExitStack() as ctx:
            build_kernel_body(ctx, tc, out_d, inp_d, bias_d)
    nc.compile()
    return nc


_CACHED_NC = None


def get_nc():
    global _CACHED_NC
    if _CACHED_NC is None:
        _CACHED_NC = build_nc()
    return _CACHED_NC


def make_in_maps(x, w_qkv, w_out, b_out):
    x = np.ascontiguousarray(np.asarray(x, dtype=np.float32)).reshape(N_CORES, HW, C)
    # [c, (t, p)] with column t*128+p = token 8p+t, bf16
    xt = np.ascontiguousarray(
        x.reshape(N_CORES, 128, NT, C).transpose(0, 3, 2, 1).reshape(N_CORES, C, HW)
    ).astype(ml_dtypes.bfloat16)
    w_qkv_bf = np.asarray(w_qkv, dtype=np.float32).astype(ml_dtypes.bfloat16)
    w_out = np.asarray(w_out, dtype=np.float32)
    b_out = np.asarray(b_out, dtype=np.float32).reshape(C)

    # woa: rows [w_out[0:32]; 0; w_out[32:64]; 0]  (heads 0, 1)
    # wob: rows [w_out[64:96]; 0; w_out[96:128]; 0]  (heads 2, 3)
    woa = np.zeros((128, C), dtype=np.float32)
    wob = np.zeros((128, C), dtype=np.float32)
    woa[0:32] = w_out[0:32]
    woa[64:96] = w_out[32:64]
    wob[0:32] = w_out[64:96]
    wob[64:96] = w_out[96:128]
    woa = woa.astype(ml_dtypes.bfloat16)
    wob = wob.astype(ml_dtypes.bfloat16)
    bias = np.ascontiguousarray(
        np.broadcast_to(b_out[None, None, :], (128, NT, C)).astype(np.float32))
    inp = np.concatenate(
        [xt, np.broadcast_to(w_qkv_bf[None], (N_CORES, C, 3 * C)),
         np.broadcast_to(woa[None], (N_CORES, 128, C)),
         np.broadcast_to(wob[None], (N_CORES, 128, C))], axis=2)
    inp = np.ascontiguousarray(inp).astype(ml_dtypes.bfloat16)
    return [
        {"inp": inp[i], "bias": bias}
        for i in range(N_CORES)
    ]


def kernel(x, w_qkv, w_out, b_out, _trace=False, _trace_kwargs=None):
    nc = get_nc()
    in_maps = make_in_maps(x, w_qkv, w_out, b_out)
    res = run_bass_kernel_spmd(
        nc, in_maps, core_ids=list(range(N_CORES)),
        trace=_trace, **(_trace_kwargs or {}),
    )
    out = np.stack([np.asarray(res.results[i]["out"]) for i in range(N_CORES)])
    out = out.reshape(8, 32, 32, 128).astype(np.float32)
    if _trace:
        kernel.last_result = res
    return out
